# revision 2
# baseline (speedup 1.0000x reference)
"""LocalityAttention TRN2 kernel.

Reference computation (per batch b of 16):
    Q = q @ Wq.T + bq; K = k @ Wk.T + bk; V = v @ Wv.T + bv          [1024, 768]
    scores = (Q @ K.T) / temperature, diag set to -1e4
    out = softmax(scores) @ V

Sharding: data-parallel over batch, 2 batches per core x 8 cores. No
collectives. Weights replicated.

Per-core device pipeline (all matmuls float32r — full PE rate at moving
dim >=256, ~tf32 operand rounding):
  - inputs fed host-transposed: qT/kT/vT [2, 768, 1024], weights W.T [d_in, e]
  - Q^T,K^T projected into [e, s] layout, V into [s, e] (natural)
  - per 128-row q-tile: scores psum [128, 1024], diagonal mask added via a
    -1e4*I tile, row max (DVE), exp with fused bias/-max and row-sum
    accumulation (ACT), PE-transpose of the attention tile, attn @ V,
    normalize by reciprocal row sum + bv on DVE, DMA out.
temperature is folded into qT and bq on the host (scores/T == (q/T)-path).
bv is added after normalization (softmax rows sum to 1, so this is exact).
"""

import numpy as np

import concourse.bacc as bacc
import concourse.mybir as mybir
from concourse.tile import TileContext
from concourse.bass_utils import run_bass_kernel_spmd
from concourse.masks import make_identity

B, S, D = 16, 1024, 768
NCORES = 8
BL = B // NCORES          # batches per core
P = 128
DC = D // P               # 6 contraction chunks
NQT = S // P              # 8 q-tiles
KCH = 512
NKC = S // KCH            # 2 k-chunks
EW = [(0, 512), (512, 256)]  # e-chunks for [*, 768] psum outputs

F32 = mybir.dt.float32
F32R = mybir.dt.float32r
AF = mybir.ActivationFunctionType
AX = mybir.AxisListType
ALU = mybir.AluOpType

_CACHE = {}


def _build():
    nc = bacc.Bacc(None, target_bir_lowering=False)
    qT = nc.declare_dram_parameter("qT", [BL, D, S], F32R, isOutput=False)
    kT = nc.declare_dram_parameter("kT", [BL, D, S], F32R, isOutput=False)
    vT = nc.declare_dram_parameter("vT", [BL, D, S], F32R, isOutput=False)
    WqT = nc.declare_dram_parameter("WqT", [D, D], F32R, isOutput=False)
    WkT = nc.declare_dram_parameter("WkT", [D, D], F32R, isOutput=False)
    WvT = nc.declare_dram_parameter("WvT", [D, D], F32R, isOutput=False)
    bq2 = nc.declare_dram_parameter("bq2", [P, DC], F32, isOutput=False)
    bk2 = nc.declare_dram_parameter("bk2", [P, DC], F32, isOutput=False)
    bvr = nc.declare_dram_parameter("bvr", [P, D], F32, isOutput=False)
    out = nc.declare_dram_parameter("out", [BL, S, D], F32, isOutput=True)

    with TileContext(nc) as tc:
        with (
            tc.tile_pool(name="const", bufs=1) as const,
            tc.tile_pool(name="big", bufs=1) as big,
            tc.tile_pool(name="stage", bufs=2) as stage,
            tc.tile_pool(name="attn", bufs=2) as attnp,
            tc.tile_pool(name="attnT", bufs=2) as attnTp,
            tc.tile_pool(name="outp", bufs=3) as outp,
            tc.tile_pool(name="stats", bufs=24) as stats,
            tc.tile_pool(name="ps_proj", bufs=2, space="PSUM") as ps_proj,
            tc.tile_pool(name="ps_sc", bufs=2, space="PSUM") as ps_sc,
            tc.tile_pool(name="ps_pv", bufs=2, space="PSUM") as ps_pv,
            tc.tile_pool(name="ps_tr", bufs=2, space="PSUM") as ps_tr,
        ):
            # ---- constants -------------------------------------------------
            wq_sb = const.tile([P, DC, D], F32R, name="wq")
            wk_sb = const.tile([P, DC, D], F32R, name="wk")
            wv_sb = const.tile([P, DC, D], F32R, name="wv")
            for w_sb, w_d in ((wq_sb, WqT), (wk_sb, WkT), (wv_sb, WvT)):
                nc.sync.dma_start(w_sb[:], w_d.ap().rearrange("(o p) e -> p o e", p=P))
            bq_sb = const.tile([P, DC], F32, name="bq")
            bk_sb = const.tile([P, DC], F32, name="bk")
            bv_sb = const.tile([P, D], F32, name="bv")
            nc.sync.dma_start(bq_sb[:], bq2.ap())
            nc.sync.dma_start(bk_sb[:], bk2.ap())
            nc.sync.dma_start(bv_sb[:], bvr.ap())

            ident_f = const.tile([P, P], F32, name="identf")
            make_identity(nc, ident_f[:])
            ident = const.tile([P, P], F32R, name="ident")
            nc.scalar.activation(ident[:], ident_f[:], AF.Copy)

            diagneg = const.tile([P, P], F32, name="diagneg")
            nc.gpsimd.memset(diagneg[:], 0.0)
            # out[x, y] = (x - y) != 0 ? in : -1e4  -> -1e4 on the diagonal
            nc.gpsimd.affine_select(
                out=diagneg[:], in_=diagneg[:],
                compare_op=ALU.not_equal, fill=-10000.0,
                base=0, pattern=[[-1, P]], channel_multiplier=1,
            )

            for b in range(BL):
                # ---- Q^T / K^T projections: [e, s] = W.T.T @ xT ------------
                QT_sb = big.tile([P, DC, S], F32R, name="QT")
                KT_sb = big.tile([P, DC, S], F32R, name="KT")
                V_sb = big.tile([P, NQT, D], F32R, name="V")
                for x_d, w_sb, b_sb, dst in (
                    (qT, wq_sb, bq_sb, QT_sb),
                    (kT, wk_sb, bk_sb, KT_sb),
                ):
                    x_t = x_d.ap()[b].rearrange("(o p) s -> p o s", p=P)
                    for sc in range(NKC):
                        st = stage.tile([P, DC, KCH], F32R, tag="stage")
                        nc.sync.dma_start(st[:], x_t[:, :, sc * KCH:(sc + 1) * KCH])
                        for ec in range(DC):
                            ps = ps_proj.tile([P, KCH], F32, name="pp")
                            for dc in range(DC):
                                nc.tensor.matmul(
                                    ps[:], w_sb[:, dc, ec * P:(ec + 1) * P],
                                    st[:, dc],
                                    start=(dc == 0), stop=(dc == DC - 1),
                                )
                            nc.scalar.activation(
                                dst[:, ec, sc * KCH:(sc + 1) * KCH], ps[:],
                                AF.Identity, bias=b_sb[:, ec:ec + 1],
                            )

                # ---- V projection: [s, e] = vT.T @ Wv.T --------------------
                v_t = vT.ap()[b].rearrange("(o p) s -> p o s", p=P)
                for sc in range(NKC):
                    st = stage.tile([P, DC, KCH], F32R, tag="stage")
                    nc.sync.dma_start(st[:], v_t[:, :, sc * KCH:(sc + 1) * KCH])
                    for st4 in range(KCH // P):
                        s_tile = sc * (KCH // P) + st4
                        for (e0, ew) in EW:
                            ps = ps_proj.tile([P, KCH], F32, name="pp")
                            for dc in range(DC):
                                nc.tensor.matmul(
                                    ps[:, :ew],
                                    st[:, dc, st4 * P:(st4 + 1) * P],
                                    wv_sb[:, dc, e0:e0 + ew],
                                    start=(dc == 0), stop=(dc == DC - 1),
                                )
                            nc.scalar.activation(
                                V_sb[:, s_tile, e0:e0 + ew], ps[:, :ew], AF.Copy,
                            )

                # ---- attention per q-tile ----------------------------------
                for qt in range(NQT):
                    pss = []
                    for kc in range(NKC):
                        ps = ps_sc.tile([P, KCH], F32, name="psc")
                        for ec in range(DC):
                            nc.tensor.matmul(
                                ps[:], QT_sb[:, ec, qt * P:(qt + 1) * P],
                                KT_sb[:, ec, kc * KCH:(kc + 1) * KCH],
                                start=(ec == 0), stop=(ec == DC - 1),
                            )
                        pss.append(ps)
                    kcd, off = divmod(qt * P, KCH)
                    nc.vector.tensor_add(
                        pss[kcd][:, off:off + P], pss[kcd][:, off:off + P],
                        diagneg[:],
                    )
                    m0 = stats.tile([P, 1], F32, tag="st")
                    m1 = stats.tile([P, 1], F32, tag="st")
                    negmax = stats.tile([P, 1], F32, tag="st")
                    nc.vector.tensor_reduce(m0[:], pss[0][:], axis=AX.X,
                                            op=ALU.max, negate=True)
                    nc.vector.tensor_reduce(m1[:], pss[1][:], axis=AX.X,
                                            op=ALU.max, negate=True)
                    nc.vector.tensor_tensor(negmax[:], m0[:], m1[:], ALU.min)

                    at = attnp.tile([P, S], F32R, tag="attn")
                    rs0 = stats.tile([P, 1], F32, tag="st")
                    rs1 = stats.tile([P, 1], F32, tag="st")
                    nc.scalar.activation(at[:, 0:KCH], pss[0][:], AF.Exp,
                                         bias=negmax[:], accum_out=rs0[:])
                    nc.scalar.activation(at[:, KCH:S], pss[1][:], AF.Exp,
                                         bias=negmax[:], accum_out=rs1[:])
                    rsum = stats.tile([P, 1], F32, tag="st")
                    rinv = stats.tile([P, 1], F32, tag="st")
                    nc.vector.tensor_add(rsum[:], rs0[:], rs1[:])
                    nc.vector.reciprocal(rinv[:], rsum[:])

                    att = attnTp.tile([P, S], F32R, tag="attnT")
                    for kc8 in range(NQT):
                        pt = ps_tr.tile([P, P], F32R, name="ptr")
                        nc.tensor.transpose(pt[:], at[:, kc8 * P:(kc8 + 1) * P],
                                            ident[:])
                        nc.scalar.activation(att[:, kc8 * P:(kc8 + 1) * P],
                                             pt[:], AF.Copy)

                    po = [ps_pv.tile([P, KCH], F32, name="ppv") for _ in EW]
                    for kc8 in range(NQT):
                        for i, (e0, ew) in enumerate(EW):
                            nc.tensor.matmul(
                                po[i][:, :ew], att[:, kc8 * P:(kc8 + 1) * P],
                                V_sb[:, kc8, e0:e0 + ew],
                                start=(kc8 == 0), stop=(kc8 == NQT - 1),
                            )
                    ou = outp.tile([P, D], F32, tag="out")
                    for i, (e0, ew) in enumerate(EW):
                        nc.vector.tensor_scalar_mul(ou[:, e0:e0 + ew],
                                                    po[i][:, :ew], rinv[:])
                    nc.vector.tensor_add(ou[:], ou[:], bv_sb[:])
                    nc.sync.dma_start(out.ap()[b, qt * P:(qt + 1) * P, :], ou[:])

    nc.finalize()
    return nc


def _get_nc():
    if "nc" not in _CACHE:
        _CACHE["nc"] = _build()
    return _CACHE["nc"]


def kernel(q, k, v, Wq, bq, Wk, bk, Wv, bv, temperature, _trace=False):
    q = np.asarray(q, dtype=np.float32)
    k = np.asarray(k, dtype=np.float32)
    v = np.asarray(v, dtype=np.float32)
    temp = float(np.asarray(temperature))

    qT = np.ascontiguousarray(np.transpose(q, (0, 2, 1)) / temp)
    kT = np.ascontiguousarray(np.transpose(k, (0, 2, 1)))
    vT = np.ascontiguousarray(np.transpose(v, (0, 2, 1)))
    WqT = np.ascontiguousarray(np.asarray(Wq, np.float32).T)
    WkT = np.ascontiguousarray(np.asarray(Wk, np.float32).T)
    WvT = np.ascontiguousarray(np.asarray(Wv, np.float32).T)
    bq2 = np.ascontiguousarray(
        (np.asarray(bq, np.float32) / temp).reshape(DC, P).T)
    bk2 = np.ascontiguousarray(np.asarray(bk, np.float32).reshape(DC, P).T)
    bvr = np.ascontiguousarray(
        np.tile(np.asarray(bv, np.float32)[None, :], (P, 1)))

    nc = _get_nc()
    in_maps = []
    for c in range(NCORES):
        sl = slice(c * BL, (c + 1) * BL)
        in_maps.append({
            "qT": qT[sl], "kT": kT[sl], "vT": vT[sl],
            "WqT": WqT, "WkT": WkT, "WvT": WvT,
            "bq2": bq2, "bk2": bk2, "bvr": bvr,
        })
    res = run_bass_kernel_spmd(nc, in_maps, list(range(NCORES)), trace=_trace)
    out = np.concatenate([res.results[c]["out"] for c in range(NCORES)], axis=0)
    if _trace:
        return out, res
    return out


# revision 6
# speedup vs baseline: 1.6643x; 1.6643x over previous
"""LocalityAttention TRN2 kernel.

Reference computation (per batch b of 16):
    Q = q @ Wq.T + bq; K = k @ Wk.T + bk; V = v @ Wv.T + bv          [1024, 768]
    scores = (Q @ K.T) / temperature, diag set to -1e4
    out = softmax(scores) @ V

Sharding: data-parallel over batch, 2 batches per core x 8 cores. No
collectives. Weights replicated.

Per-core device pipeline (all matmuls float32r — full PE rate at moving
dim >=256, ~tf32 operand rounding):
  - inputs fed host-transposed: qT/kT/vT [2, 768, 1024], weights W.T [d_in, e]
  - Q^T,K^T projected into [e, s] layout, V into [s, e] (natural)
  - per 128-row q-tile: scores psum [128, 1024], diagonal mask added via a
    -1e4*I tile, row max (DVE), exp with fused bias/-max and row-sum
    accumulation (ACT), PE-transpose of the attention tile, attn @ V,
    normalize by reciprocal row sum + bv on DVE, DMA out.
temperature is folded into qT and bq on the host (scores/T == (q/T)-path).
bv is added after normalization (softmax rows sum to 1, so this is exact).
"""

import numpy as np

import concourse.bacc as bacc
import concourse.mybir as mybir
from concourse.tile import TileContext
from concourse.bass_utils import run_bass_kernel_spmd
from concourse.masks import make_identity

B, S, D = 16, 1024, 768
NCORES = 8
BL = B // NCORES          # batches per core
P = 128
DC = D // P               # 6 contraction chunks
NQT = S // P              # 8 q-tiles
KCH = 512
NKC = S // KCH            # 2 k-chunks
EW = [(0, 512), (512, 256)]  # e-chunks for [*, 768] psum outputs

F32 = mybir.dt.float32
F32R = mybir.dt.float32r
AF = mybir.ActivationFunctionType
AX = mybir.AxisListType
ALU = mybir.AluOpType

_CACHE = {}


def _build():
    nc = bacc.Bacc(None, target_bir_lowering=False)
    qT = nc.declare_dram_parameter("qT", [BL, D, S], F32R, isOutput=False)
    kT = nc.declare_dram_parameter("kT", [BL, D, S], F32R, isOutput=False)
    vT = nc.declare_dram_parameter("vT", [BL, D, S], F32R, isOutput=False)
    WqT = nc.declare_dram_parameter("WqT", [D, D], F32R, isOutput=False)
    WkT = nc.declare_dram_parameter("WkT", [D, D], F32R, isOutput=False)
    WvT = nc.declare_dram_parameter("WvT", [D, D], F32R, isOutput=False)
    bq2 = nc.declare_dram_parameter("bq2", [P, DC], F32, isOutput=False)
    bk2 = nc.declare_dram_parameter("bk2", [P, DC], F32, isOutput=False)
    bvr = nc.declare_dram_parameter("bvr", [P, D], F32, isOutput=False)
    out = nc.declare_dram_parameter("out", [BL, S, D], F32, isOutput=True)

    with TileContext(nc) as tc:
        with (
            tc.tile_pool(name="const", bufs=1) as const,
            tc.tile_pool(name="big", bufs=1) as big,
            tc.tile_pool(name="stage", bufs=2) as stage,
            tc.tile_pool(name="attn", bufs=2) as attnp,
            tc.tile_pool(name="attnT", bufs=2) as attnTp,
            tc.tile_pool(name="outp", bufs=3) as outp,
            tc.tile_pool(name="stats", bufs=24) as stats,
            tc.tile_pool(name="ps_mm", bufs=2, space="PSUM") as ps_mm,
            tc.tile_pool(name="ps_sc", bufs=4, space="PSUM") as ps_sc,
            tc.tile_pool(name="ps_tr", bufs=2, space="PSUM") as ps_tr,
        ):
            # ---- constants -------------------------------------------------
            wq_sb = const.tile([P, DC, D], F32R, name="wq")
            wk_sb = const.tile([P, DC, D], F32R, name="wk")
            wv_sb = const.tile([P, DC, D], F32R, name="wv")

            bq_sb = const.tile([P, DC], F32, name="bq")
            bk_sb = const.tile([P, DC], F32, name="bk")
            bv_sb = const.tile([P, D], F32, name="bv")
            nc.sync.dma_start(bq_sb[:], bq2.ap())
            nc.sync.dma_start(bk_sb[:], bk2.ap())
            nc.sync.dma_start(bv_sb[:], bvr.ap())

            ident_f = const.tile([P, P], F32, name="identf")
            make_identity(nc, ident_f[:])
            ident = const.tile([P, P], F32R, name="ident")
            nc.scalar.activation(ident[:], ident_f[:], AF.Copy)

            diagneg = const.tile([P, P], F32, name="diagneg")
            nc.gpsimd.memset(diagneg[:], 0.0)
            # out[x, y] = (x - y) != 0 ? in : -1e4  -> -1e4 on the diagonal
            nc.gpsimd.affine_select(
                out=diagneg[:], in_=diagneg[:],
                compare_op=ALU.not_equal, fill=-10000.0,
                base=0, pattern=[[-1, P]], channel_multiplier=1,
            )

            for b in range(BL):
                # ---- Q^T / K^T projections: [e, s] = W.T.T @ xT ------------
                QT_sb = big.tile([P, DC, S], F32R, name="QT")
                KT_sb = big.tile([P, DC, S], F32R, name="KT")
                V_sb = big.tile([P, NQT, D], F32R, name="V")
                for x_d, w_sb, w_dram, b_sb, dst in (
                    (qT, wq_sb, WqT, bq_sb, QT_sb),
                    (kT, wk_sb, WkT, bk_sb, KT_sb),
                ):
                    if b == 0:
                        w_t = w_dram.ap().rearrange("(o p) e -> p o e", p=P)
                        for dc in range(DC):
                            nc.sync.dma_start(w_sb[:, dc], w_t[:, dc])
                    x_t = x_d.ap()[b].rearrange("(o p) s -> p o s", p=P)
                    for sc in range(NKC):
                        st = stage.tile([P, DC, KCH], F32R, tag="stage")
                        nc.sync.dma_start(st[:], x_t[:, :, sc * KCH:(sc + 1) * KCH])
                        for ec in range(DC):
                            ps = ps_mm.tile([P, KCH], F32, tag="pp", name="pp")
                            for dc in range(DC):
                                nc.tensor.matmul(
                                    ps[:], w_sb[:, dc, ec * P:(ec + 1) * P],
                                    st[:, dc],
                                    start=(dc == 0), stop=(dc == DC - 1),
                                )
                            nc.scalar.activation(
                                dst[:, ec, sc * KCH:(sc + 1) * KCH], ps[:],
                                AF.Identity, bias=b_sb[:, ec:ec + 1],
                            )

                # ---- V projection: [s, e] = vT.T @ Wv.T --------------------
                if b == 0:
                    w_t = WvT.ap().rearrange("(o p) e -> p o e", p=P)
                    for dc in range(DC):
                        nc.sync.dma_start(wv_sb[:, dc], w_t[:, dc])
                v_t = vT.ap()[b].rearrange("(o p) s -> p o s", p=P)
                for sc in range(NKC):
                    st = stage.tile([P, DC, KCH], F32R, tag="stage")
                    nc.sync.dma_start(st[:], v_t[:, :, sc * KCH:(sc + 1) * KCH])
                    for st4 in range(KCH // P):
                        s_tile = sc * (KCH // P) + st4
                        for (e0, ew) in EW:
                            ps = ps_mm.tile([P, KCH], F32, tag="pp", name="pp")
                            for dc in range(DC):
                                nc.tensor.matmul(
                                    ps[:, :ew],
                                    st[:, dc, st4 * P:(st4 + 1) * P],
                                    wv_sb[:, dc, e0:e0 + ew],
                                    start=(dc == 0), stop=(dc == DC - 1),
                                )
                            nc.scalar.activation(
                                V_sb[:, s_tile, e0:e0 + ew], ps[:, :ew], AF.Copy,
                            )

                # ---- attention per q-tile ----------------------------------
                for qt in range(NQT):
                    pss = []
                    for kc in range(NKC):
                        ps = ps_sc.tile([P, KCH], F32, name="psc")
                        for ec in range(DC):
                            nc.tensor.matmul(
                                ps[:], QT_sb[:, ec, qt * P:(qt + 1) * P],
                                KT_sb[:, ec, kc * KCH:(kc + 1) * KCH],
                                start=(ec == 0), stop=(ec == DC - 1),
                            )
                        pss.append(ps)
                    kcd, off = divmod(qt * P, KCH)
                    nc.vector.tensor_add(
                        pss[kcd][:, off:off + P], pss[kcd][:, off:off + P],
                        diagneg[:],
                    )
                    m0 = stats.tile([P, 1], F32, tag="st")
                    m1 = stats.tile([P, 1], F32, tag="st")
                    negmax = stats.tile([P, 1], F32, tag="st")
                    nc.vector.tensor_reduce(m0[:], pss[0][:], axis=AX.X,
                                            op=ALU.max, negate=True)
                    nc.vector.tensor_reduce(m1[:], pss[1][:], axis=AX.X,
                                            op=ALU.max, negate=True)
                    nc.vector.tensor_tensor(negmax[:], m0[:], m1[:], ALU.min)

                    at = attnp.tile([P, S], F32R, tag="attn")
                    rs0 = stats.tile([P, 1], F32, tag="st")
                    rs1 = stats.tile([P, 1], F32, tag="st")
                    nc.scalar.activation(at[:, 0:KCH], pss[0][:], AF.Exp,
                                         bias=negmax[:], accum_out=rs0[:])
                    nc.scalar.activation(at[:, KCH:S], pss[1][:], AF.Exp,
                                         bias=negmax[:], accum_out=rs1[:])
                    rsum = stats.tile([P, 1], F32, tag="st")
                    rinv = stats.tile([P, 1], F32, tag="st")
                    nc.vector.tensor_add(rsum[:], rs0[:], rs1[:])
                    nc.vector.reciprocal(rinv[:], rsum[:])

                    att = attnTp.tile([P, S], F32R, tag="attnT")
                    for g in range(NQT // 4):
                        pt = ps_tr.tile([P, 4 * P], F32R, name="ptr")
                        for j in range(4):
                            kc8 = g * 4 + j
                            nc.tensor.transpose(pt[:, j * P:(j + 1) * P],
                                                at[:, kc8 * P:(kc8 + 1) * P],
                                                ident[:])
                        nc.scalar.activation(att[:, g * 4 * P:(g + 1) * 4 * P],
                                             pt[:], AF.Copy)

                    po = [ps_mm.tile([P, KCH], F32, tag="pp", name="ppv") for _ in EW]
                    for kc8 in range(NQT):
                        for i, (e0, ew) in enumerate(EW):
                            nc.tensor.matmul(
                                po[i][:, :ew], att[:, kc8 * P:(kc8 + 1) * P],
                                V_sb[:, kc8, e0:e0 + ew],
                                start=(kc8 == 0), stop=(kc8 == NQT - 1),
                            )
                    ou = outp.tile([P, D], F32, tag="out")
                    for i, (e0, ew) in enumerate(EW):
                        nc.vector.tensor_scalar_mul(ou[:, e0:e0 + ew],
                                                    po[i][:, :ew], rinv[:])
                    nc.vector.tensor_add(ou[:], ou[:], bv_sb[:])
                    nc.sync.dma_start(out.ap()[b, qt * P:(qt + 1) * P, :], ou[:])

    nc.finalize()
    return nc


def _get_nc():
    if "nc" not in _CACHE:
        _CACHE["nc"] = _build()
    return _CACHE["nc"]


def kernel(q, k, v, Wq, bq, Wk, bk, Wv, bv, temperature, _trace=False):
    q = np.asarray(q, dtype=np.float32)
    k = np.asarray(k, dtype=np.float32)
    v = np.asarray(v, dtype=np.float32)
    temp = float(np.asarray(temperature))

    qT = np.ascontiguousarray(np.transpose(q, (0, 2, 1)) / temp)
    kT = np.ascontiguousarray(np.transpose(k, (0, 2, 1)))
    vT = np.ascontiguousarray(np.transpose(v, (0, 2, 1)))
    WqT = np.ascontiguousarray(np.asarray(Wq, np.float32).T)
    WkT = np.ascontiguousarray(np.asarray(Wk, np.float32).T)
    WvT = np.ascontiguousarray(np.asarray(Wv, np.float32).T)
    bq2 = np.ascontiguousarray(
        (np.asarray(bq, np.float32) / temp).reshape(DC, P).T)
    bk2 = np.ascontiguousarray(np.asarray(bk, np.float32).reshape(DC, P).T)
    bvr = np.ascontiguousarray(
        np.tile(np.asarray(bv, np.float32)[None, :], (P, 1)))

    nc = _get_nc()
    in_maps = []
    for c in range(NCORES):
        sl = slice(c * BL, (c + 1) * BL)
        in_maps.append({
            "qT": qT[sl], "kT": kT[sl], "vT": vT[sl],
            "WqT": WqT, "WkT": WkT, "WvT": WvT,
            "bq2": bq2, "bk2": bk2, "bvr": bvr,
        })
    res = run_bass_kernel_spmd(nc, in_maps, list(range(NCORES)), trace=_trace)
    out = np.concatenate([res.results[c]["out"] for c in range(NCORES)], axis=0)
    if _trace:
        return out, res
    return out
